# revision 18
# baseline (speedup 1.0000x reference)
"""Bass/Tile TRN2 kernel for nn_BaseModel_20925080666480 (ragged_sequence).

Pipeline per core (data-parallel over batch, 8 rows/core):
  1. Stream hiddens (bf16).
  2. Ragged subword->word mean pooling fused with the 3-layer mean: for each
     layer tile an accumulating matmul against an indicator matrix S built
     on-device (iota == ids), then scaled by 1/(3*cnt).  f32 accumulate in
     PSUM, so the only precision loss is the bf16 input quantization.
  3. Input projection X @ W_ih^T (+bias) for both LSTM directions into a
     padded gate layout (i@0:20, f@32:52, o@64:84, g@96:116 of 128).
  4. 256-step fused BiLSTM scan: fwd+bwd as 16 lanes of one chain;
     G_in preloaded into PSUM, recurrent matmuls accumulate on top.
  5. PE-transpose epilogue to [b, t, 2H] layout.
"""

import numpy as np

B, T_SUB, W, D = 64, 512, 256, 768
H = 20
CAP_DIM = 10
IN_DIM = D + CAP_DIM  # 778
NCORES = 8
BL = B // NCORES  # 8 batch rows per core
LANES = 2 * BL    # 16 scan lanes (fwd + bwd)
KDIM = IN_DIM + 1  # 779, bias folded as last row
NTCH = T_SUB // 128  # 4 t-chunks
NWCH = 4             # 4 w-chunks of 64 words
WCH = W // NWCH      # 64
NDCH = D // 128      # 6 d-chunks

# padded gate layout offsets within 128 partitions
GI, GF, GO, GG = 0, 32, 64, 96

_STATE = {}


def _gate_pad_cols(m):
    """m: [..., 80] pytorch gate order i,f,g,o -> [..., 128] padded i,f,o,g."""
    out = np.zeros(m.shape[:-1] + (128,), np.float32)
    out[..., GI:GI + H] = m[..., 0 * H:1 * H]
    out[..., GF:GF + H] = m[..., 1 * H:2 * H]
    out[..., GG:GG + H] = m[..., 2 * H:3 * H]
    out[..., GO:GO + H] = m[..., 3 * H:4 * H]
    return out


def _build(nz_blocks):
    """Build + compile the Bass module (fast path, diagonal nz pattern)."""
    import concourse.bacc as bacc
    import concourse.mybir as mybir
    import concourse.tile as tile
    from contextlib import ExitStack

    f32 = mybir.dt.float32
    bf16 = mybir.dt.bfloat16
    AF = mybir.ActivationFunctionType
    MUL = mybir.AluOpType.mult
    EQ = mybir.AluOpType.is_equal

    nc = bacc.Bacc("TRN2", target_bir_lowering=False, debug=False,
                   enable_asserts=False)

    hid = nc.dram_tensor("hid", [3, BL, T_SUB + 1, D], bf16, kind="ExternalInput")
    idsf = nc.dram_tensor("idsf", [BL * T_SUB, 1], f32, kind="ExternalInput")
    recipb = nc.dram_tensor("recipb", [BL, 128, W], f32, kind="ExternalInput")
    iota = nc.dram_tensor("iota", [128, W], f32, kind="ExternalInput")
    capt = nc.dram_tensor("capt", [11, W * BL], f32, kind="ExternalInput")
    wih = nc.dram_tensor("wih", [2, KDIM, 128], f32, kind="ExternalInput")
    whh = nc.dram_tensor("whh", [H, 256], f32, kind="ExternalInput")
    ident = nc.dram_tensor("ident", [H, H], f32, kind="ExternalInput")
    outp = nc.dram_tensor("out", [BL, W, 2 * H], f32, kind="ExternalOutput")

    NCOL = W * BL  # 2048 columns of X^T, col = w*BL + b

    with tile.TileContext(nc) as tc, ExitStack() as ctx:
        cst = ctx.enter_context(tc.tile_pool(name="cst", bufs=1))
        ldp = ctx.enter_context(tc.tile_pool(name="ldp", bufs=3))
        xtp = ctx.enter_context(tc.tile_pool(name="xtp", bufs=1))
        gp = ctx.enter_context(tc.tile_pool(name="gp", bufs=1))
        smal = ctx.enter_context(tc.tile_pool(name="smal", bufs=4))
        osbp = ctx.enter_context(tc.tile_pool(name="osbp", bufs=4))
        pps = ctx.enter_context(tc.tile_pool(name="pps", bufs=2, space="PSUM"))
        imps = ctx.enter_context(tc.tile_pool(name="imps", bufs=2, space="PSUM"))
        scps = ctx.enter_context(tc.tile_pool(name="scps", bufs=4, space="PSUM"))

        # ---- constants ----
        iota_t = cst.tile([128, W], f32, tag="iota")
        nc.sync.dma_start(iota_t[:], iota[:])
        capt_t = cst.tile([11, NCOL], f32, tag="capt")
        nc.sync.dma_start(capt_t[:], capt[:])
        wih_t = []  # [dir][kchunk]
        for d in range(2):
            tiles = []
            for j in range(6):
                t = cst.tile([128, 128], f32, tag=f"wih{d}_{j}",
                             name=f"wih{d}_{j}")
                nc.sync.dma_start(t[:], wih[d, j * 128:(j + 1) * 128, :])
                tiles.append(t)
            t = cst.tile([11, 128], f32, tag=f"wih{d}_6", name=f"wih{d}_6")
            nc.sync.dma_start(t[:], wih[d, 768:KDIM, :])
            tiles.append(t)
            wih_t.append(tiles)
        whh_t = cst.tile([H, 256], f32, tag="whh")
        nc.sync.dma_start(whh_t[:], whh[:])
        ident_t = cst.tile([H, H], f32, tag="ident")
        nc.sync.dma_start(ident_t[:], ident[:])
        hzero = cst.tile([H, BL], f32, tag="hzero")
        nc.vector.memset(hzero[:], 0.0)
        scl = cst.tile([128, 1], f32, tag="scl")
        nc.vector.memset(scl[:], 1.0)
        nc.vector.memset(scl[GG:GG + H, :], 2.0)
        bneg = cst.tile([128, 1], f32, tag="bneg")
        nc.vector.memset(bneg[:], -1.0)
        c_t = cst.tile([H, LANES], f32, tag="cstate")
        nc.vector.memset(c_t[:], 0.0)
        recip_t = {}
        for b in range(BL):
            rt = cst.tile([128, W], f32, tag=f"recip{b}", name=f"recip{b}")
            nc.gpsimd.dma_start(rt[:], recipb[b, :, :])
            recip_t[b] = rt

        # persistent big tiles
        xt = [xtp.tile([128, NCOL], f32, tag=f"xt{j}", name=f"xt{j}")
              for j in range(NDCH)]
        g2 = gp.tile([128, 2 * NCOL], f32, tag="g2")  # fwd 0:2048, bwd 2048:4096
        hs2 = gp.tile([H, 2 * W * BL], f32, tag="hs2")  # col = dir*W*BL + t*BL + b

        # ---- phase A+B: stream, pool, project (pipelined per t-chunk) ----
        assert all(tuple(nz_blocks[c]) == (c,) for c in range(NWCH))
        for T in [0, 3, 1, 2]:
            c = T  # diagonal: this t-chunk completes w-chunk c
            w0 = c * WCH
            n0 = c * WCH * BL
            for b in range(BL):
                lds = []
                row0 = 1 + T * 128
                dma_eng = [nc.sync, nc.gpsimd, nc.scalar]
                for l in range(3):
                    lt = ldp.tile([128, D], bf16, tag=f"ld{l}", name=f"ld{l}")
                    dma_eng[l].dma_start(lt[:], hid[l, b, row0:row0 + 128, :])
                    lds.append(lt)
                idt = smal.tile([128, 1], f32, tag="ids")
                nc.gpsimd.dma_start(
                    idt[:], idsf[b * T_SUB + T * 128:b * T_SUB + (T + 1) * 128, :])
                smt = smal.tile([128, W], bf16, tag="smat")
                nc.vector.tensor_scalar(smt[:], iota_t[:], idt[:], None, EQ)
                for j in range(NDCH):
                    ps = pps.tile([128, WCH], f32, tag="pps")
                    for l in range(3):
                        nc.tensor.matmul(ps[:], lds[l][:, j * 128:(j + 1) * 128],
                                         smt[:, w0:w0 + WCH],
                                         start=(l == 0), stop=(l == 2))
                    # scale by 1/(3*cnt), scatter to X^T cols w*BL+b
                    nc.vector.tensor_tensor(
                        xt[j][:, n0 + b:n0 + WCH * BL:BL],
                        ps[:], recip_t[b][:, w0:w0 + WCH], MUL)
            # input projection for this w-chunk, both directions
            for d in range(2):
                pim = imps.tile([128, WCH * BL], f32, tag="imps")
                for j in range(NDCH):
                    nc.tensor.matmul(pim[:], wih_t[d][j][:],
                                     xt[j][:, n0:n0 + WCH * BL],
                                     start=(j == 0), stop=False)
                nc.tensor.matmul(pim[:], wih_t[d][6][:],
                                 capt_t[:, n0:n0 + WCH * BL],
                                 start=False, stop=True)
                nc.vector.tensor_copy(
                    g2[:, d * NCOL + n0:d * NCOL + n0 + WCH * BL], pim[:])

        # ---- BiLSTM scan: 256 steps, 16 lanes ----
        import dataclasses
        WB = W * BL
        g2ap = g2[:]
        hs2ap = hs2[:]
        for s in range(W):
            p = scps.tile([128, LANES], f32, tag="sc")
            # one preload op for both dirs: 2-level free AP (fwd blk s, bwd blk 255-s)
            gsrc = dataclasses.replace(
                g2ap, offset=g2ap.offset + s * BL,
                ap=[g2ap.ap[0], [NCOL + (W - 1 - 2 * s) * BL, 2], [1, BL]])
            nc.vector.tensor_copy(p[:, 0:LANES], gsrc)
            hf_prev = hzero[:] if s == 0 else hs2[:, (s - 1) * BL:s * BL]
            hb_prev = (hzero[:] if s == 0
                       else hs2[:, WB + (W - s) * BL:WB + (W - s + 1) * BL])
            nc.tensor.matmul(p[:, 0:BL], whh_t[:, 0:128], hf_prev,
                             start=False, stop=True, skip_group_check=True)
            nc.tensor.matmul(p[:, BL:LANES], whh_t[:, 128:256], hb_prev,
                             start=False, stop=True, skip_group_check=True)
            # sigmoid in-place in PSUM (g rows get sigma(2g) via scale AP);
            # downstream DVE ops read gate slices straight from PSUM so all
            # SBUF APs stay at start partition 0 (walrus verifier rule).
            nc.scalar.activation(p[0:GG + H, :], p[0:GG + H, :], AF.Sigmoid,
                                 scale=scl[0:GG + H, :])
            tg = smal.tile([H, LANES], f32, tag="tg")
            nc.scalar.activation(tg[:], p[GG:GG + H, :], AF.Identity,
                                 bias=bneg[0:H, :], scale=2.0)
            tmp = smal.tile([H, LANES], f32, tag="tmp")
            nc.vector.tensor_tensor(tmp[:], p[GI:GI + H, :], tg[:], MUL)
            nc.vector.tensor_tensor(c_t[:], c_t[:], p[GF:GF + H, :], MUL)
            nc.vector.tensor_add(c_t[:], c_t[:], tmp[:])
            tch = smal.tile([H, LANES], f32, tag="tch")
            nc.scalar.activation(tch[:], c_t[:], AF.Tanh)
            # single h write for both dirs (fwd blk s, bwd blk W+(255-s))
            hout = dataclasses.replace(
                hs2ap, offset=hs2ap.offset + s * BL,
                ap=[hs2ap.ap[0], [WB + (W - 1 - 2 * s) * BL, 2], [1, BL]])
            nc.vector.tensor_tensor(hout, p[GO:GO + H, :], tch[:], MUL)

        # ---- epilogue: transpose to [b, t, 2H] ----
        for b in range(BL):
            for TW in range(W // 128):
                osb = osbp.tile([128, 2 * H], f32, tag="osb")
                tpf = scps.tile([128, H], f32, tag="sc")
                nc.tensor.transpose(
                    tpf[:],
                    hs2[:, TW * 128 * BL + b:TW * 128 * BL + b + 127 * BL + 1:BL],
                    ident_t[:])
                nc.scalar.copy(osb[:, 0:H], tpf[:])
                tpb = scps.tile([128, H], f32, tag="sc")
                nc.tensor.transpose(
                    tpb[:],
                    hs2[:, WB + TW * 128 * BL + b:WB + TW * 128 * BL + b + 127 * BL + 1:BL],
                    ident_t[:])
                nc.scalar.copy(osb[:, H:2 * H], tpb[:])
                nc.sync.dma_start(outp[b, TW * 128:(TW + 1) * 128, :], osb[:])

    nc.compile()
    return nc


def _is_diag(nz_blocks):
    return all(tuple(nz_blocks[c]) == (c,) for c in range(NWCH))


def _prep_inputs(inputs):
    """Host-side prep. Returns (in_maps, nz_blocks)."""
    import ml_dtypes
    bf = ml_dtypes.bfloat16

    hiddens = np.asarray(inputs["hiddens"], np.float32)
    bert2toks = np.asarray(inputs["bert2toks"]).astype(np.int64)
    cap_inds = np.asarray(inputs["cap_inds"]).astype(np.int64)
    cap_table = np.asarray(inputs["cap_table"], np.float32)
    w_ih_f = np.asarray(inputs["w_ih_f"], np.float32)
    w_hh_f = np.asarray(inputs["w_hh_f"], np.float32)
    b_f = np.asarray(inputs["b_f"], np.float32)
    w_ih_b = np.asarray(inputs["w_ih_b"], np.float32)
    w_hh_b = np.asarray(inputs["w_hh_b"], np.float32)
    b_b = np.asarray(inputs["b_b"], np.float32)

    hid_bf = hiddens.astype(bf)

    cnt = np.zeros((B, W), np.int64)
    for b in range(B):
        cnt[b] = np.bincount(bert2toks[b], minlength=W)[:W]
    recip = np.where(cnt > 0, 1.0 / (3.0 * np.maximum(cnt, 1)), 0.0).astype(np.float32)
    recipb = np.broadcast_to(recip[:, None, :], (B, 128, W))

    idsf = bert2toks.astype(np.float32)
    iota_v = np.broadcast_to(np.arange(W, dtype=np.float32), (128, W))

    cap_emb = cap_table[cap_inds]  # [B, W, 10]

    wih_all = np.zeros((2, KDIM, 128), np.float32)
    for d, (wi, bb) in enumerate(((w_ih_f, b_f), (w_ih_b, b_b))):
        wih_all[d, :IN_DIM, :] = _gate_pad_cols(wi.T.reshape(IN_DIM, 80))
        wih_all[d, IN_DIM, :] = _gate_pad_cols(bb[None, :])[0]
    whh_all = np.zeros((H, 256), np.float32)
    whh_all[:, 0:128] = _gate_pad_cols(w_hh_f.T.reshape(H, 80))
    whh_all[:, 128:256] = _gate_pad_cols(w_hh_b.T.reshape(H, 80))

    tch_idx = np.repeat(np.arange(NTCH), 128)
    nz = [[] for _ in range(NWCH)]
    for T in [0, 3, 1, 2]:
        mask = tch_idx == T
        ids_T = bert2toks[:, mask]
        for c in range(NWCH):
            if np.any((ids_T >= c * WCH) & (ids_T < (c + 1) * WCH)):
                if T not in nz[c]:
                    nz[c].append(T)

    in_maps = []
    for cc in range(NCORES):
        b0 = cc * BL
        ce = cap_emb[b0:b0 + BL]  # [8, 256, 10]
        capt = np.ones((11, W * BL), np.float32)
        capt[0:10] = ce.transpose(2, 1, 0).reshape(10, W * BL)
        in_maps.append({
            "hid": np.ascontiguousarray(hid_bf[:, b0:b0 + BL]),
            "idsf": np.ascontiguousarray(idsf[b0:b0 + BL]).reshape(BL * T_SUB, 1),
            "recipb": np.ascontiguousarray(recipb[b0:b0 + BL]),
            "iota": np.ascontiguousarray(iota_v),
            "capt": capt,
            "wih": wih_all,
            "whh": whh_all,
            "ident": np.eye(H, dtype=np.float32),
        })
    return in_maps, tuple(tuple(x) for x in nz)


def _kernel_numpy(inputs):
    """General fallback (non-diagonal segment patterns): plain numpy."""
    hiddens = np.asarray(inputs["hiddens"], np.float32)
    bert2toks = np.asarray(inputs["bert2toks"]).astype(np.int64)
    cap_inds = np.asarray(inputs["cap_inds"]).astype(np.int64)
    cap_table = np.asarray(inputs["cap_table"], np.float32)
    means = hiddens.mean(axis=0)
    sub = means[:, 1:T_SUB + 1]
    flat_ids = (bert2toks + np.arange(B)[:, None] * W).reshape(-1)
    sums = np.zeros((B * W, D), np.float32)
    np.add.at(sums, flat_ids, sub.reshape(B * T_SUB, D))
    cnts = np.zeros((B * W, 1), np.float32)
    np.add.at(cnts, flat_ids, 1.0)
    with np.errstate(divide="ignore", invalid="ignore"):
        word_h = (sums / cnts).reshape(B, W, D)
    x = np.concatenate([word_h, cap_table[cap_inds]], axis=-1)

    def lstm(wi, whh, bb, rev):
        g_in = x.reshape(B * W, -1) @ wi.T + bb
        g_in = g_in.reshape(B, W, 4 * H)
        h = np.zeros((B, H), np.float32)
        c = np.zeros((B, H), np.float32)
        hs = np.empty((B, W, H), np.float32)
        sig = lambda v: 1.0 / (1.0 + np.exp(-v))
        for t in (range(W - 1, -1, -1) if rev else range(W)):
            g = g_in[:, t] + h @ whh.T
            i, f, gg, o = np.split(g, 4, 1)
            c = sig(f) * c + sig(i) * np.tanh(gg)
            h = sig(o) * np.tanh(c)
            hs[:, t] = h
        return hs

    return np.concatenate([
        lstm(np.asarray(inputs["w_ih_f"], np.float32),
             np.asarray(inputs["w_hh_f"], np.float32),
             np.asarray(inputs["b_f"], np.float32), False),
        lstm(np.asarray(inputs["w_ih_b"], np.float32),
             np.asarray(inputs["w_hh_b"], np.float32),
             np.asarray(inputs["b_b"], np.float32), True),
    ], axis=-1)


def _get_nc(nz_blocks):
    key = ("nc", nz_blocks)
    if key not in _STATE:
        _STATE[key] = _build(nz_blocks)
    return _STATE[key]


def kernel(**inputs) -> np.ndarray:
    in_maps, nz_blocks = _prep_inputs(inputs)
    if not _is_diag(nz_blocks):
        return _kernel_numpy(inputs)
    from concourse.bass_utils import run_bass_kernel_spmd
    nc = _get_nc(nz_blocks)
    res = run_bass_kernel_spmd(nc, in_maps, list(range(NCORES)))
    return np.concatenate([res.results[c]["out"] for c in range(NCORES)], axis=0)


# revision 21
# speedup vs baseline: 1.0055x; 1.0055x over previous
"""Bass/Tile TRN2 kernel for nn_BaseModel_20925080666480 (ragged_sequence).

Pipeline per core (data-parallel over batch, 8 rows/core):
  1. Stream hiddens (bf16).
  2. Ragged subword->word mean pooling fused with the 3-layer mean: for each
     layer tile an accumulating matmul against an indicator matrix S built
     on-device (iota == ids), then scaled by 1/(3*cnt).  f32 accumulate in
     PSUM, so the only precision loss is the bf16 input quantization.
  3. Input projection X @ W_ih^T (+bias) for both LSTM directions into a
     padded gate layout (i@0:20, f@32:52, o@64:84, g@96:116 of 128).
  4. 256-step fused BiLSTM scan: fwd+bwd as 16 lanes of one chain;
     G_in preloaded into PSUM, recurrent matmuls accumulate on top.
  5. PE-transpose epilogue to [b, t, 2H] layout.
"""

import numpy as np

B, T_SUB, W, D = 64, 512, 256, 768
H = 20
CAP_DIM = 10
IN_DIM = D + CAP_DIM  # 778
NCORES = 8
BL = B // NCORES  # 8 batch rows per core
LANES = 2 * BL    # 16 scan lanes (fwd + bwd)
KDIM = IN_DIM + 1  # 779, bias folded as last row
NTCH = T_SUB // 128  # 4 t-chunks
NWCH = 4             # 4 w-chunks of 64 words
WCH = W // NWCH      # 64
NDCH = D // 128      # 6 d-chunks

# padded gate layout offsets within 128 partitions
GI, GF, GO, GG = 0, 32, 64, 96

_STATE = {}


def _gate_pad_cols(m):
    """m: [..., 80] pytorch gate order i,f,g,o -> [..., 128] padded i,f,o,g."""
    out = np.zeros(m.shape[:-1] + (128,), np.float32)
    out[..., GI:GI + H] = m[..., 0 * H:1 * H]
    out[..., GF:GF + H] = m[..., 1 * H:2 * H]
    out[..., GG:GG + H] = m[..., 2 * H:3 * H]
    out[..., GO:GO + H] = m[..., 3 * H:4 * H]
    return out


def _build(nz_blocks):
    """Build + compile the Bass module (fast path, diagonal nz pattern)."""
    import concourse.bacc as bacc
    import concourse.mybir as mybir
    import concourse.tile as tile
    from contextlib import ExitStack

    f32 = mybir.dt.float32
    bf16 = mybir.dt.bfloat16
    AF = mybir.ActivationFunctionType
    MUL = mybir.AluOpType.mult
    EQ = mybir.AluOpType.is_equal

    nc = bacc.Bacc("TRN2", target_bir_lowering=False, debug=False,
                   enable_asserts=False)

    hidm = nc.dram_tensor("hidm", [BL, NTCH, 128, 3 * D], bf16,
                          kind="ExternalInput")
    idsc = nc.dram_tensor("idsc", [128, NTCH * BL], f32, kind="ExternalInput")
    recipc = nc.dram_tensor("recipc", [128, BL * W], f32, kind="ExternalInput")
    iota = nc.dram_tensor("iota", [128, W], f32, kind="ExternalInput")
    capt = nc.dram_tensor("capt", [11, W * BL], f32, kind="ExternalInput")
    wihw = nc.dram_tensor("wihw", [2, 128, 6 * 128], f32, kind="ExternalInput")
    wih6 = nc.dram_tensor("wih6", [2, 11, 128], f32, kind="ExternalInput")
    whh = nc.dram_tensor("whh", [H, 256], f32, kind="ExternalInput")
    ident = nc.dram_tensor("ident", [H, H], f32, kind="ExternalInput")
    outp = nc.dram_tensor("out", [BL, W, 2 * H], f32, kind="ExternalOutput")

    NCOL = W * BL  # 2048 columns of X^T, col = w*BL + b

    with tile.TileContext(nc) as tc, ExitStack() as ctx:
        cst = ctx.enter_context(tc.tile_pool(name="cst", bufs=1))
        ldp = ctx.enter_context(tc.tile_pool(name="ldp", bufs=3))
        xtp = ctx.enter_context(tc.tile_pool(name="xtp", bufs=1))
        gp = ctx.enter_context(tc.tile_pool(name="gp", bufs=1))
        smal = ctx.enter_context(tc.tile_pool(name="smal", bufs=4))
        osbp = ctx.enter_context(tc.tile_pool(name="osbp", bufs=4))
        pps = ctx.enter_context(tc.tile_pool(name="pps", bufs=2, space="PSUM"))
        imps = ctx.enter_context(tc.tile_pool(name="imps", bufs=2, space="PSUM"))
        scps = ctx.enter_context(tc.tile_pool(name="scps", bufs=4, space="PSUM"))

        # ---- constants ----
        iota_t = cst.tile([128, W], f32, tag="iota")
        nc.sync.dma_start(iota_t[:], iota[:])
        capt_t = cst.tile([11, NCOL], f32, tag="capt")
        nc.sync.dma_start(capt_t[:], capt[:])
        wih_t = []  # [dir] -> (wide [128, 768], last [11, 128])
        for d in range(2):
            wide = cst.tile([128, 6 * 128], f32, tag=f"wihw{d}",
                            name=f"wihw{d}")
            nc.scalar.dma_start(wide[:], wihw[d, :, :])
            last = cst.tile([11, 128], f32, tag=f"wih{d}_6", name=f"wih{d}_6")
            nc.sync.dma_start(last[:], wih6[d, :, :])
            wih_t.append((wide, last))
        whh_t = cst.tile([H, 256], f32, tag="whh")
        nc.sync.dma_start(whh_t[:], whh[:])
        ident_t = cst.tile([H, H], f32, tag="ident")
        nc.sync.dma_start(ident_t[:], ident[:])
        hzero = cst.tile([H, BL], f32, tag="hzero")
        nc.vector.memset(hzero[:], 0.0)
        scl = cst.tile([128, 1], f32, tag="scl")
        nc.vector.memset(scl[:], 1.0)
        nc.vector.memset(scl[GG:GG + H, :], 2.0)
        bneg = cst.tile([128, 1], f32, tag="bneg")
        nc.vector.memset(bneg[:], -1.0)
        c_t = cst.tile([H, LANES], f32, tag="cstate")
        nc.vector.memset(c_t[:], 0.0)
        recip_all = cst.tile([128, BL * W], f32, tag="recipall")
        nc.sync.dma_start(recip_all[:], recipc[:])
        ids_all = cst.tile([128, NTCH * BL], f32, tag="idsall")
        nc.scalar.dma_start(ids_all[:], idsc[:])

        # persistent big tiles
        xt = [xtp.tile([128, NCOL], f32, tag=f"xt{j}", name=f"xt{j}")
              for j in range(NDCH)]
        g2 = gp.tile([128, 2 * NCOL], f32, tag="g2")  # fwd 0:2048, bwd 2048:4096
        hs2 = gp.tile([H, 2 * W * BL], f32, tag="hs2")  # col = dir*W*BL + t*BL + b

        # ---- phase A+B: stream, pool, project (pipelined per t-chunk) ----
        assert all(tuple(nz_blocks[c]) == (c,) for c in range(NWCH))
        for T in [0, 3, 1, 2]:
            c = T  # diagonal: this t-chunk completes w-chunk c
            w0 = c * WCH
            n0 = c * WCH * BL
            for b in range(BL):
                ld3 = ldp.tile([128, 3 * D], bf16, tag="ld3", name="ld3")
                (nc.sync if (T * BL + b) % 2 == 0 else nc.scalar).dma_start(
                    ld3[:], hidm[b, T, :, :])
                smt = smal.tile([128, W], bf16, tag="smat")
                nc.vector.tensor_scalar(
                    smt[:], iota_t[:], ids_all[:, T * BL + b:T * BL + b + 1],
                    None, EQ)
                for j in range(NDCH):
                    ps = pps.tile([128, WCH], f32, tag="pps")
                    for l in range(3):
                        nc.tensor.matmul(
                            ps[:], ld3[:, l * D + j * 128:l * D + (j + 1) * 128],
                            smt[:, w0:w0 + WCH],
                            start=(l == 0), stop=(l == 2))
                    # scale by 1/(3*cnt), scatter to X^T cols w*BL+b
                    nc.vector.tensor_tensor(
                        xt[j][:, n0 + b:n0 + WCH * BL:BL],
                        ps[:], recip_all[:, b * W + w0:b * W + w0 + WCH], MUL)
            # input projection for this w-chunk, both directions
            for d in range(2):
                pim = imps.tile([128, WCH * BL], f32, tag="imps")
                wide, last = wih_t[d]
                for j in range(NDCH):
                    nc.tensor.matmul(pim[:], wide[:, j * 128:(j + 1) * 128],
                                     xt[j][:, n0:n0 + WCH * BL],
                                     start=(j == 0), stop=False)
                nc.tensor.matmul(pim[:], last[:],
                                 capt_t[:, n0:n0 + WCH * BL],
                                 start=False, stop=True)
                nc.vector.tensor_copy(
                    g2[:, d * NCOL + n0:d * NCOL + n0 + WCH * BL], pim[:])

        # ---- BiLSTM scan: 256 steps, 16 lanes ----
        import dataclasses
        WB = W * BL
        g2ap = g2[:]
        hs2ap = hs2[:]
        for s in range(W):
            p = scps.tile([128, LANES], f32, tag="sc")
            # one preload op for both dirs: 2-level free AP (fwd blk s, bwd blk 255-s)
            gsrc = dataclasses.replace(
                g2ap, offset=g2ap.offset + s * BL,
                ap=[g2ap.ap[0], [NCOL + (W - 1 - 2 * s) * BL, 2], [1, BL]])
            nc.vector.tensor_copy(p[:, 0:LANES], gsrc)
            hf_prev = hzero[:] if s == 0 else hs2[:, (s - 1) * BL:s * BL]
            hb_prev = (hzero[:] if s == 0
                       else hs2[:, WB + (W - s) * BL:WB + (W - s + 1) * BL])
            nc.tensor.matmul(p[:, 0:BL], whh_t[:, 0:128], hf_prev,
                             start=False, stop=True, skip_group_check=True)
            nc.tensor.matmul(p[:, BL:LANES], whh_t[:, 128:256], hb_prev,
                             start=False, stop=True, skip_group_check=True)
            # sigmoid in-place in PSUM (g rows get sigma(2g) via scale AP);
            # downstream DVE ops read gate slices straight from PSUM so all
            # SBUF APs stay at start partition 0 (walrus verifier rule).
            nc.scalar.activation(p[0:GG + H, :], p[0:GG + H, :], AF.Sigmoid,
                                 scale=scl[0:GG + H, :])
            tg = smal.tile([H, LANES], f32, tag="tg")
            nc.scalar.activation(tg[:], p[GG:GG + H, :], AF.Identity,
                                 bias=bneg[0:H, :], scale=2.0)
            tmp = smal.tile([H, LANES], f32, tag="tmp")
            nc.vector.tensor_tensor(tmp[:], p[GI:GI + H, :], tg[:], MUL)
            nc.vector.tensor_tensor(c_t[:], c_t[:], p[GF:GF + H, :], MUL)
            nc.vector.tensor_add(c_t[:], c_t[:], tmp[:])
            tch = smal.tile([H, LANES], f32, tag="tch")
            nc.scalar.activation(tch[:], c_t[:], AF.Tanh)
            # single h write for both dirs (fwd blk s, bwd blk W+(255-s))
            hout = dataclasses.replace(
                hs2ap, offset=hs2ap.offset + s * BL,
                ap=[hs2ap.ap[0], [WB + (W - 1 - 2 * s) * BL, 2], [1, BL]])
            nc.vector.tensor_tensor(hout, p[GO:GO + H, :], tch[:], MUL)

        # ---- epilogue: transpose to [b, t, 2H] ----
        for b in range(BL):
            for TW in range(W // 128):
                osb = osbp.tile([128, 2 * H], f32, tag="osb")
                tpf = scps.tile([128, H], f32, tag="sc")
                nc.tensor.transpose(
                    tpf[:],
                    hs2[:, TW * 128 * BL + b:TW * 128 * BL + b + 127 * BL + 1:BL],
                    ident_t[:])
                nc.scalar.copy(osb[:, 0:H], tpf[:])
                tpb = scps.tile([128, H], f32, tag="sc")
                nc.tensor.transpose(
                    tpb[:],
                    hs2[:, WB + TW * 128 * BL + b:WB + TW * 128 * BL + b + 127 * BL + 1:BL],
                    ident_t[:])
                nc.scalar.copy(osb[:, H:2 * H], tpb[:])
                nc.sync.dma_start(outp[b, TW * 128:(TW + 1) * 128, :], osb[:])

    nc.compile()
    return nc


def _is_diag(nz_blocks):
    return all(tuple(nz_blocks[c]) == (c,) for c in range(NWCH))


def _prep_inputs(inputs):
    """Host-side prep. Returns (in_maps, nz_blocks)."""
    import ml_dtypes
    bf = ml_dtypes.bfloat16

    hiddens = np.asarray(inputs["hiddens"], np.float32)
    bert2toks = np.asarray(inputs["bert2toks"]).astype(np.int64)
    cap_inds = np.asarray(inputs["cap_inds"]).astype(np.int64)
    cap_table = np.asarray(inputs["cap_table"], np.float32)
    w_ih_f = np.asarray(inputs["w_ih_f"], np.float32)
    w_hh_f = np.asarray(inputs["w_hh_f"], np.float32)
    b_f = np.asarray(inputs["b_f"], np.float32)
    w_ih_b = np.asarray(inputs["w_ih_b"], np.float32)
    w_hh_b = np.asarray(inputs["w_hh_b"], np.float32)
    b_b = np.asarray(inputs["b_b"], np.float32)

    hid_bf = hiddens.astype(bf)

    cnt = np.zeros((B, W), np.int64)
    for b in range(B):
        cnt[b] = np.bincount(bert2toks[b], minlength=W)[:W]
    recip = np.where(cnt > 0, 1.0 / (3.0 * np.maximum(cnt, 1)), 0.0).astype(np.float32)

    idsf = bert2toks.astype(np.float32)
    iota_v = np.broadcast_to(np.arange(W, dtype=np.float32), (128, W))

    cap_emb = cap_table[cap_inds]  # [B, W, 10]

    wih_all = np.zeros((2, KDIM, 128), np.float32)
    for d, (wi, bb) in enumerate(((w_ih_f, b_f), (w_ih_b, b_b))):
        wih_all[d, :IN_DIM, :] = _gate_pad_cols(wi.T.reshape(IN_DIM, 80))
        wih_all[d, IN_DIM, :] = _gate_pad_cols(bb[None, :])[0]
    whh_all = np.zeros((H, 256), np.float32)
    whh_all[:, 0:128] = _gate_pad_cols(w_hh_f.T.reshape(H, 80))
    whh_all[:, 128:256] = _gate_pad_cols(w_hh_b.T.reshape(H, 80))

    tch_idx = np.repeat(np.arange(NTCH), 128)
    nz = [[] for _ in range(NWCH)]
    for T in [0, 3, 1, 2]:
        mask = tch_idx == T
        ids_T = bert2toks[:, mask]
        for c in range(NWCH):
            if np.any((ids_T >= c * WCH) & (ids_T < (c + 1) * WCH)):
                if T not in nz[c]:
                    nz[c].append(T)

    # wide weight layout: wihw[d, p, j*128+m] = wih_all[d, j*128+p, m]
    wihw = np.zeros((2, 128, 6 * 128), np.float32)
    for j in range(6):
        wihw[:, :, j * 128:(j + 1) * 128] = wih_all[:, j * 128:(j + 1) * 128, :]
    wih6 = np.ascontiguousarray(wih_all[:, 768:KDIM, :])

    # hiddens merged per (b, tchunk): [BL, NTCH, 128, 3*D], skipping CLS
    hs_sub = hid_bf[:, :, 1:T_SUB + 1, :]  # [3, B, T_SUB, D]
    hs_sub = hs_sub.reshape(3, B, NTCH, 128, D)

    in_maps = []
    for cc in range(NCORES):
        b0 = cc * BL
        ce = cap_emb[b0:b0 + BL]  # [8, 256, 10]
        capt = np.ones((11, W * BL), np.float32)
        capt[0:10] = ce.transpose(2, 1, 0).reshape(10, W * BL)
        hm = np.ascontiguousarray(
            hs_sub[:, b0:b0 + BL].transpose(1, 2, 3, 0, 4).reshape(
                BL, NTCH, 128, 3 * D))
        ic = np.empty((128, NTCH * BL), np.float32)
        rc = np.empty((128, BL * W), np.float32)
        for bb in range(BL):
            for T in range(NTCH):
                ic[:, T * BL + bb] = idsf[b0 + bb, T * 128:(T + 1) * 128]
            rc[:, bb * W:(bb + 1) * W] = recip[b0 + bb][None, :]
        in_maps.append({
            "hidm": hm,
            "idsc": ic,
            "recipc": rc,
            "iota": np.ascontiguousarray(iota_v),
            "capt": capt,
            "wihw": wihw,
            "wih6": wih6,
            "whh": whh_all,
            "ident": np.eye(H, dtype=np.float32),
        })
    return in_maps, tuple(tuple(x) for x in nz)


def _kernel_numpy(inputs):
    """General fallback (non-diagonal segment patterns): plain numpy."""
    hiddens = np.asarray(inputs["hiddens"], np.float32)
    bert2toks = np.asarray(inputs["bert2toks"]).astype(np.int64)
    cap_inds = np.asarray(inputs["cap_inds"]).astype(np.int64)
    cap_table = np.asarray(inputs["cap_table"], np.float32)
    means = hiddens.mean(axis=0)
    sub = means[:, 1:T_SUB + 1]
    flat_ids = (bert2toks + np.arange(B)[:, None] * W).reshape(-1)
    sums = np.zeros((B * W, D), np.float32)
    np.add.at(sums, flat_ids, sub.reshape(B * T_SUB, D))
    cnts = np.zeros((B * W, 1), np.float32)
    np.add.at(cnts, flat_ids, 1.0)
    with np.errstate(divide="ignore", invalid="ignore"):
        word_h = (sums / cnts).reshape(B, W, D)
    x = np.concatenate([word_h, cap_table[cap_inds]], axis=-1)

    def lstm(wi, whh, bb, rev):
        g_in = x.reshape(B * W, -1) @ wi.T + bb
        g_in = g_in.reshape(B, W, 4 * H)
        h = np.zeros((B, H), np.float32)
        c = np.zeros((B, H), np.float32)
        hs = np.empty((B, W, H), np.float32)
        sig = lambda v: 1.0 / (1.0 + np.exp(-v))
        for t in (range(W - 1, -1, -1) if rev else range(W)):
            g = g_in[:, t] + h @ whh.T
            i, f, gg, o = np.split(g, 4, 1)
            c = sig(f) * c + sig(i) * np.tanh(gg)
            h = sig(o) * np.tanh(c)
            hs[:, t] = h
        return hs

    return np.concatenate([
        lstm(np.asarray(inputs["w_ih_f"], np.float32),
             np.asarray(inputs["w_hh_f"], np.float32),
             np.asarray(inputs["b_f"], np.float32), False),
        lstm(np.asarray(inputs["w_ih_b"], np.float32),
             np.asarray(inputs["w_hh_b"], np.float32),
             np.asarray(inputs["b_b"], np.float32), True),
    ], axis=-1)


def _get_nc(nz_blocks):
    key = ("nc", nz_blocks)
    if key not in _STATE:
        _STATE[key] = _build(nz_blocks)
    return _STATE[key]


def kernel(**inputs) -> np.ndarray:
    in_maps, nz_blocks = _prep_inputs(inputs)
    if not _is_diag(nz_blocks):
        return _kernel_numpy(inputs)
    from concourse.bass_utils import run_bass_kernel_spmd
    nc = _get_nc(nz_blocks)
    res = run_bass_kernel_spmd(nc, in_maps, list(range(NCORES)))
    return np.concatenate([res.results[c]["out"] for c in range(NCORES)], axis=0)
